# revision 57
# baseline (speedup 1.0000x reference)
"""ARGenerator TRN2 kernel (single 1024-wide chunk, streaming pipeline).

Math (per batch row b):
  h1 = relu(x @ W1.T + b1); h2 = relu(h1 @ W2.T + b2)
  mlp = tanh(h2 @ W3.T + b3)
  ar[t] = noise[t] + sum_i c[i] * ar[t-1-i]  (zero-init, t >= 7; 0 for t < 7)
  out = mlp + ar

The AR recurrence is linear time-invariant -> ar = conv(noise_masked, h)
with h the (geometrically decaying) impulse response, truncated at
(nb-1)*128 taps.  The conv becomes nb banded 128x128 Toeplitz matmuls
per output time-tile, fully parallel over time.

Layout strategy (pure data parallel over 8 cores, B_shard = 1024):
  bf16 (fp8 for x/W1/W3), TRANSPOSED activation layout [feature/time on
  partitions, batch on free dim].  The host pre-TILES x and noise to
  [128, tiles*batch] so every DMA descriptor moves 4KB contiguous per
  partition; output comes back the same way.

Pipeline: ONE chunk of CW=1024 (the full shard).  Matmuls write
512-wide PSUM bank halves (matmul cannot cross a PSUM bank), but the
scalar ACT (tanh + per-partition b3 bias) and the DVE add read the full
[128, 1024] tile in ONE instruction, amortizing the (N+352)/1.2ns fixed
overhead.  L1 uses DoubleRow fp8 matmuls (2 k-tiles per instruction).
PSUM is exactly 2 pools x 2 bufs x 2 banks = 8 banks.

DMA: the sync queue carries the ordered critical stream W1 -> x ->
noise (per-queue FIFO + pool backpressure keep noise JIT and mostly
behind x); gpsimd carries consts + W3 early and the stores later;
scalar is left completely clear for the ACT chain.

Engine budget per t-tile (1024 batch): Tensor 6x216=1296ns,
Scalar 1147, DVE 1220, wire (noise in + out) ~1300ns.
Per-core wire: 0.5MB W1 + 0.6MB consts/W3 + 4MB xT + 8MB nT + 8MB out.
"""

import numpy as np
import ml_dtypes

import concourse.bass as bass
import concourse.tile as tile
import concourse.mybir as mybir
from concourse import bacc

F32 = mybir.dt.float32
BF16 = mybir.dt.bfloat16
F8 = mybir.dt.float8e4
BF16_NP = ml_dtypes.bfloat16
F8_NP = ml_dtypes.float8_e4m3
W1_SCALE = 64.0
DR = mybir.MatmulPerfMode.DoubleRow


def impulse_response(c, s_out, tail_tol=1e-4):
    """Return (h, nb) with (nb-1)*128 taps covering the response."""
    AR = len(c)
    c = np.asarray(c, np.float64)
    h = np.zeros(s_out, np.float64)
    h[0] = 1.0
    for j in range(1, s_out):
        acc = 0.0
        for i in range(AR):
            if j - 1 - i >= 0:
                acc += c[i] * h[j - 1 - i]
        h[j] = acc
    L = 128
    while L < s_out and np.abs(h[L:]).sum() > tail_tol:
        L += 128
    # nb = number of 128-wide band blocks per output tile: the in-tile block
    # (j=0) plus one per preceding input tile the L-tap history reaches into.
    return h, L // 128 + 1


def band_blocks(h, nb):
    """Hb [128, nb*128]: block jj (for input-tile offset j = jj - (nb-1))
    has Hb[k_rel, t_rel] = h[t_rel - k_rel - 128*j] (0 <= lag < (nb-1)*128)."""
    L = (nb - 1) * 128
    a = np.arange(128)[:, None]   # k_rel
    b = np.arange(128)[None, :]   # t_rel
    blocks = []
    for jj in range(nb):
        j = jj - (nb - 1)
        lag = b - a - 128 * j
        m = (lag >= 0) & (lag < L)
        blk = np.where(m, np.take(np.pad(h[:L], (0, 1)), np.clip(lag, 0, L)), 0.0)
        blocks.append(blk)
    return np.concatenate(blocks, axis=1)


def host_prepare(W1, b1, W2, b2, W3, b3, ar_coef, S_IN, S_OUT, H):
    """Small device tensors in exactly the SBUF layout used."""
    n_s = S_IN // 128
    # W1l[p, k*H + h] = W1[h, k*128 + p]  (lhsT k-tiles for layer 1)
    W1l = np.ascontiguousarray(
        W1.reshape(H, n_s, 128).transpose(2, 1, 0)).reshape(128, -1)
    h, nb = impulse_response(ar_coef, S_OUT)
    b3m = b3.reshape(S_OUT // 128, 128).T                    # [128, n_t]
    # merge the small consts into two blobs (one descriptor each): many
    # small DMAs serialize on their completion semaphores and drag the
    # critical load prefix.
    wb = np.concatenate(
        [np.ascontiguousarray(W2.T), band_blocks(h, nb)], axis=1)
    bias = np.concatenate(
        [b1.reshape(H, 1), b2.reshape(H, 1), b3m], axis=1)
    return {
        "W1l": (W1l * W1_SCALE).astype(F8_NP),  # fp8, scaled into e4m3 range
        "W3l": (np.ascontiguousarray(W3.T) * W1_SCALE).astype(F8_NP),
        "WbB": wb.astype(BF16_NP),              # [128, H + nb*128]: W2l | Hb
        "bias": np.ascontiguousarray(bias, np.float32),  # [128, 2+n_t]
    }, nb


def build_kernel(B_shard, S_IN, S_OUT, H, nb):
    P = 128
    CW = B_shard                  # 1024: one chunk, the full shard
    HW = CW // 2                  # 512: matmul free width (one PSUM bank)
    assert H == P and CW == 1024 and nb == 2
    n_s = S_IN // P               # 32 input k-tiles
    n_t = S_OUT // P              # 32 output t-tiles

    nc = bacc.Bacc(trn_type="TRN2", target_bir_lowering=False, debug=False)

    # x/noise/out are HOST-PRE-TILED to [128, tiles*batch] so every DMA
    # descriptor moves 4KB contiguous per partition (DMA throughput is
    # packet-size bound: ~135GB/s at 1KB chunks vs ~400+ at >=2KB).
    xT_d = nc.dram_tensor("xT", [P, n_s, CW], F8, kind="ExternalInput").ap()
    nT_d = nc.dram_tensor("nT", [P, n_t, CW], BF16,
                          kind="ExternalInput").ap()
    W1_d = nc.dram_tensor("W1l", [P, n_s * H], F8, kind="ExternalInput").ap()
    W3_d = nc.dram_tensor("W3l", [H, S_OUT], F8, kind="ExternalInput").ap()
    Wb_d = nc.dram_tensor("WbB", [P, H + nb * P], BF16,
                          kind="ExternalInput").ap()
    bias_d = nc.dram_tensor("bias", [P, 2 + n_t], F32,
                            kind="ExternalInput").ap()
    out_d = nc.dram_tensor("outT", [P, n_t, CW], BF16,
                           kind="ExternalOutput").ap()

    with tile.TileContext(nc) as tc:
        with tc.tile_pool(name="const", bufs=1) as cpool:
            # the tiny bias blob goes FIRST on sync: the first descriptor
            # of a queue pays the DMA descriptor-path warm-up (~1-2us), so
            # spend it on 17KB instead of on W1/x.
            biass = cpool.tile([P, 2 + n_t], F32, tag="bias")
            nc.sync.dma_start(biass[:], bias_d[:])
            b1s = biass[:, 0:1]
            b2s = biass[:, 1:2]
            b3s = biass[:, 2:]
            W1s = cpool.tile([P, n_s * H], F8, tag="w1")
            nc.sync.dma_start(W1s[:], W1_d[:])

            def W1dr(kk):     # k-pair kk -> [128, 2, H] fp8 lhsT
                return W1s[:, 2 * kk * H:(2 * kk + 2) * H].rearrange(
                    "p (a b) -> p a b", a=2)

            Wbs = cpool.tile([P, H + nb * P], BF16, tag="wb")
            nc.gpsimd.dma_start(Wbs[:], Wb_d[:])
            W2s = Wbs[:, :H]
            Hbs = Wbs[:, H:]

            with (
                tc.tile_pool(name="warm", bufs=1) as wpool,
                tc.tile_pool(name="xT", bufs=4) as xTp,
                tc.tile_pool(name="nT", bufs=4) as nTp,
                tc.tile_pool(name="act", bufs=2) as actp,
                tc.tile_pool(name="th", bufs=3) as thp,
                tc.tile_pool(name="outT", bufs=3) as outp,
                tc.tile_pool(name="psA", bufs=2, space="PSUM") as psA,
                tc.tile_pool(name="psB", bufs=2, space="PSUM") as psB,
            ):
                # ---- x: 4 descriptors of 1MB, ALL on sync right behind
                # W1.  Sync wins wire arbitration, so any x placed on
                # another queue gets starved once sync moves on to noise;
                # the whole ordered critical stream must ride one queue.
                xts = []
                for g in range(n_s // 8):
                    t = xTp.tile([P, 8, CW], F8, tag="xt", name=f"xt{g}")
                    nc.sync.dma_start(t[:],
                                      xT_d[:, g * 8:(g + 1) * 8, :])
                    xts.append(t)
                W3s = cpool.tile([H, S_OUT], F8, tag="w3")
                nc.gpsimd.dma_start(W3s[:], W3_d[:])

                def xdr(kk, h):   # k-pair kk, batch-half h -> [128,2,512] f8
                    g, j = kk // 4, 2 * (kk % 4)
                    return xts[g][:, j:j + 2, h * HW:(h + 1) * HW]

                # ---- noise: 16 descriptors of 512KB (2 t-tiles) on sync
                # strictly behind x; pool backpressure (bufs=3 = 1.5MB
                # eager window) bounds how much noise can interleave into
                # the x phase while the in-loop JIT issue keeps the reload
                # pipeline going without steady-state stalls.
                ntm = []

                def load_n2(g):
                    t = nTp.tile([P, 2, CW], BF16, tag="nt", name=f"nt{g}")
                    nc.sync.dma_start(t[:], nT_d[:, g * 2:(g + 1) * 2, :])
                    ntm.append(t)

                for g in range(4):
                    load_n2(g)

                def nt(m, h):
                    return ntm[m // 2][:, m % 2, h * HW:(h + 1) * HW]

                # ---- PE warm-up: the HAM clock gate defaults the PE array
                # to 1.2 GHz and releases 2.4 GHz only after ~3.4us of
                # sustained matmul activity; warm on a zeroed scratch tile
                # while the first x groups stream in.
                wsrc = wpool.tile([P, 4 * P], BF16, tag="wsrc")
                nc.vector.memset(wsrc[:], 0.0)
                wsnk = wpool.tile([P, 4], F32, tag="wsnk")
                psw = psB.tile([P, CW], F32, tag="ps", name="psw")
                for i in range(10):
                    nc.tensor.matmul(psw[:, :HW], wsrc[:, :P], wsrc[:])
                nc.vector.tensor_copy(wsnk[:], psw[:, :4])

                # ---- L1: 16 DoubleRow fp8 matmuls per batch half.
                psh1 = psA.tile([H, CW], F32, tag="psA", name="psh1")
                for kk in range(n_s // 2):
                    for h in range(2):
                        nc.tensor.matmul(
                            psh1[:, h * HW:(h + 1) * HW], W1dr(kk), xdr(kk, h),
                            start=(kk == 0), stop=(kk == n_s // 2 - 1),
                            perf_mode=DR,
                        )
                h1T = actp.tile([H, CW], BF16, tag="act", name="h1T")
                # scale undoes the x64 put on W1 to lift fp8 denormals
                nc.scalar.activation(
                    h1T[:], psh1[:], mybir.ActivationFunctionType.Relu,
                    bias=b1s, scale=1.0 / W1_SCALE,
                )
                psh2 = psA.tile([H, CW], F32, tag="psA", name="psh2")
                for h in range(2):
                    nc.tensor.matmul(psh2[:, h * HW:(h + 1) * HW], W2s,
                                     h1T[:, h * HW:(h + 1) * HW])
                h2T = actp.tile([H, CW], BF16, tag="act", name="h2T")
                nc.scalar.activation(
                    h2T[:], psh2[:], mybir.ActivationFunctionType.Relu,
                    bias=b2s,
                )

                # ---- t-loop: conv + W3 matmuls (512-wide halves, grouped
                # by stationary), 1024-wide tanh ACT and DVE add, 2-tile
                # merged stores on gpsimd.
                for m in range(n_t):
                    if m % 2 == 0 and m // 2 + 4 < n_t // 2:
                        load_n2(m // 2 + 4)
                    jlist = [j for j in range(-(nb - 1), 1) if m + j >= 0]
                    psc = psB.tile([P, CW], F32, tag="ps", name=f"psc{m}")
                    for i, j in enumerate(jlist):
                        jj = j + nb - 1
                        for h in range(2):
                            nc.tensor.matmul(
                                psc[:, h * HW:(h + 1) * HW],
                                Hbs[:, jj * P:(jj + 1) * P], nt(m + j, h),
                                start=(i == 0), stop=(i == len(jlist) - 1),
                            )
                    psm = psA.tile([P, CW], F32, tag="psA", name=f"psm{m}")
                    for h in range(2):
                        nc.tensor.matmul(
                            psm[:, h * HW:(h + 1) * HW],
                            W3s[:, m * P:(m + 1) * P],
                            h2T[:, h * HW:(h + 1) * HW],
                        )
                    if m % 2 == 0:
                        ot = outp.tile([P, 2, CW], BF16, tag="ot",
                                       name=f"ot{m // 2}")
                    if m == n_t - 1:
                        # finer-grained tail: per-half ACT/add, then store
                        # the two final tiles separately so the drain after
                        # the last matmul is shorter.
                        th = thp.tile([P, CW], BF16, tag="th")
                        for h in range(2):
                            hs = slice(h * HW, (h + 1) * HW)
                            nc.scalar.activation(
                                th[:, hs], psm[:, hs],
                                mybir.ActivationFunctionType.Tanh,
                                bias=b3s[:, m:m + 1], scale=1.0 / W1_SCALE,
                            )
                            nc.vector.tensor_add(ot[:, 1, hs], th[:, hs],
                                                 psc[:, hs])
                            if h == 0:
                                nc.gpsimd.dma_start(
                                    out_d[:, m - 1:m, :], ot[:, 0:1, :])
                        nc.gpsimd.dma_start(out_d[:, m:m + 1, :],
                                            ot[:, 1:2, :])
                        continue
                    th = thp.tile([P, CW], BF16, tag="th")
                    nc.scalar.activation(
                        th[:], psm[:], mybir.ActivationFunctionType.Tanh,
                        bias=b3s[:, m:m + 1], scale=1.0 / W1_SCALE,
                    )
                    nc.vector.tensor_add(ot[:, m % 2, :], th[:], psc[:])
                    if m % 2 == 1:
                        # all stores on gpsimd: on sync they would delay the
                        # in-loop noise reloads in queue-FIFO order.
                        nc.gpsimd.dma_start(out_d[:, m - 1:m + 1, :], ot[:])

    nc.compile()
    return nc


# ---------------------------------------------------------------------------
# Self-contained kernel() entry point (the graded contract).
# ---------------------------------------------------------------------------

N_CORES = 8
_B, _S_IN, _S_OUT, _H, _AR = 8192, 4096, 4096, 128, 7

_CACHE = {}


def _prep_and_build(inputs):
    dev, nb = host_prepare(
        np.asarray(inputs["W1"], np.float32), np.asarray(inputs["b1"], np.float32),
        np.asarray(inputs["W2"], np.float32), np.asarray(inputs["b2"], np.float32),
        np.asarray(inputs["W3"], np.float32), np.asarray(inputs["b3"], np.float32),
        np.asarray(inputs["ar_coef"], np.float32),
        _S_IN, _S_OUT, _H,
    )
    B_total = inputs["x"].shape[0]
    B_shard = B_total // N_CORES
    key = (B_shard, nb)
    if key not in _CACHE:
        _CACHE[key] = build_kernel(B_shard, _S_IN, _S_OUT, _H, nb)
    return _CACHE[key], dev, B_shard


def _tiled_T(a, dt):
    """[B_shard, S] -> [128, (S//128)*B_shard]: host pre-tiling so device
    DMA descriptors move long contiguous per-partition chunks."""
    s = a.shape[1]
    aT = a.astype(dt).T                           # [S, B_shard]
    return np.ascontiguousarray(
        aT.reshape(s // 128, 128, a.shape[0]).transpose(1, 0, 2))


def _in_maps(inputs, dev, B_shard):
    x = np.asarray(inputs["x"], np.float32)
    noise_m = np.asarray(inputs["noise"], np.float32).copy()
    noise_m[:, :_AR] = 0.0
    maps = []
    for c in range(N_CORES):
        sl = slice(c * B_shard, (c + 1) * B_shard)
        m = {"xT": _tiled_T(x[sl], F8_NP),
             "nT": _tiled_T(noise_m[sl], BF16_NP)}
        m.update(dev)
        maps.append(m)
    return maps


def kernel(**inputs):
    nc, dev, B_shard = _prep_and_build(inputs)
    maps = _in_maps(inputs, dev, B_shard)
    import concourse.bass_utils as bass_utils

    res = bass_utils.run_bass_kernel_spmd(
        nc, maps, core_ids=list(range(N_CORES)), trace=False
    )
    shards = []
    for c in range(N_CORES):
        o = np.asarray(res.results[c]["outT"])    # [128, n_t, B] bf16
        s_out = o.shape[1] * 128
        shards.append(o.transpose(1, 0, 2).reshape(s_out, -1).T)
    return np.concatenate(shards, axis=0).astype(np.float32)


def run_traced(inputs):
    """Profiled run (NTFF -> neuron-profile) for the local test harness."""
    import contextlib
    import ctypes
    import sys as _sys
    import types as _types

    so = "/opt/axon/libaxon_pjrt.so"
    if "antenv.axon_hooks" not in _sys.modules:
        try:
            lib2 = ctypes.CDLL(so)
            lib2.axon_start_nrt_profile.argtypes = [
                ctypes.POINTER(ctypes.c_int64), ctypes.c_size_t]
            lib2.axon_start_nrt_profile.restype = ctypes.c_int64
            lib2.axon_stop_nrt_profile.argtypes = [ctypes.c_char_p]
            lib2.axon_stop_nrt_profile.restype = ctypes.c_int64

            @contextlib.contextmanager
            def _hook(output_dir, device_ids):
                import jax
                jax.devices()
                if device_ids:
                    ids_arr = (ctypes.c_int64 * len(device_ids))(*device_ids)
                    rc = lib2.axon_start_nrt_profile(ids_arr, len(device_ids))
                else:
                    rc = lib2.axon_start_nrt_profile(None, 0)
                if rc != 0:
                    raise RuntimeError(f"axon_start_nrt_profile rc={rc}")
                try:
                    yield
                finally:
                    lib2.axon_stop_nrt_profile(str(output_dir).encode())

            mod = _types.ModuleType("antenv.axon_hooks")
            mod.get_axon_ntff_profile_hook = lambda: _hook
            mod.set_axon_ntff_profile_hook = lambda h: None
            _sys.modules["antenv.axon_hooks"] = mod
        except OSError:
            pass
    import concourse.bass_utils as bass_utils
    bass_utils.upload_artifacts = lambda tmpdir: tmpdir

    nc, dev, B_shard = _prep_and_build(inputs)
    maps = _in_maps(inputs, dev, B_shard)
    return bass_utils.run_bass_kernel_spmd(
        nc, maps, core_ids=list(range(N_CORES)), trace=True, trace_cores=[0]
    )


# revision 58
# speedup vs baseline: 1.0246x; 1.0246x over previous
"""ARGenerator TRN2 kernel (single 1024-wide chunk, streaming pipeline).

Math (per batch row b):
  h1 = relu(x @ W1.T + b1); h2 = relu(h1 @ W2.T + b2)
  mlp = tanh(h2 @ W3.T + b3)
  ar[t] = noise[t] + sum_i c[i] * ar[t-1-i]  (zero-init, t >= 7; 0 for t < 7)
  out = mlp + ar

The AR recurrence is linear time-invariant -> ar = conv(noise_masked, h)
with h the (geometrically decaying) impulse response, truncated at
(nb-1)*128 taps.  The conv becomes nb banded 128x128 Toeplitz matmuls
per output time-tile, fully parallel over time.

Layout strategy (pure data parallel over 8 cores, B_shard = 1024):
  bf16 (fp8 for x/W1/W3), TRANSPOSED activation layout [feature/time on
  partitions, batch on free dim].  The host pre-TILES x and noise to
  [128, tiles*batch] so every DMA descriptor moves 4KB contiguous per
  partition; output comes back the same way.

Pipeline: ONE chunk of CW=1024 (the full shard).  Matmuls write
512-wide PSUM bank halves (matmul cannot cross a PSUM bank), but the
scalar ACT (tanh + per-partition b3 bias) and the DVE add read the full
[128, 1024] tile in ONE instruction, amortizing the (N+352)/1.2ns fixed
overhead.  L1 uses DoubleRow fp8 matmuls (2 k-tiles per instruction).
PSUM is exactly 2 pools x 2 bufs x 2 banks = 8 banks.

DMA: the sync queue carries the ordered critical stream W1 -> x ->
noise (per-queue FIFO + pool backpressure keep noise JIT and mostly
behind x); gpsimd carries consts + W3 early and the stores later;
scalar is left completely clear for the ACT chain.

Engine budget per t-tile (1024 batch): Tensor 6x216=1296ns,
Scalar 1147, DVE 1220, wire (noise in + out) ~1300ns.
Per-core wire: 0.5MB W1 + 0.6MB consts/W3 + 4MB xT + 8MB nT + 8MB out.
"""

import numpy as np
import ml_dtypes

import concourse.bass as bass
import concourse.tile as tile
import concourse.mybir as mybir
from concourse import bacc

F32 = mybir.dt.float32
BF16 = mybir.dt.bfloat16
F8 = mybir.dt.float8e4
BF16_NP = ml_dtypes.bfloat16
F8_NP = ml_dtypes.float8_e4m3
W1_SCALE = 64.0
DR = mybir.MatmulPerfMode.DoubleRow


def impulse_response(c, s_out, tail_tol=1e-4):
    """Return (h, nb) with (nb-1)*128 taps covering the response."""
    AR = len(c)
    c = np.asarray(c, np.float64)
    h = np.zeros(s_out, np.float64)
    h[0] = 1.0
    for j in range(1, s_out):
        acc = 0.0
        for i in range(AR):
            if j - 1 - i >= 0:
                acc += c[i] * h[j - 1 - i]
        h[j] = acc
    L = 128
    while L < s_out and np.abs(h[L:]).sum() > tail_tol:
        L += 128
    # nb = number of 128-wide band blocks per output tile: the in-tile block
    # (j=0) plus one per preceding input tile the L-tap history reaches into.
    return h, L // 128 + 1


def band_blocks(h, nb):
    """Hb [128, nb*128]: block jj (for input-tile offset j = jj - (nb-1))
    has Hb[k_rel, t_rel] = h[t_rel - k_rel - 128*j] (0 <= lag < (nb-1)*128)."""
    L = (nb - 1) * 128
    a = np.arange(128)[:, None]   # k_rel
    b = np.arange(128)[None, :]   # t_rel
    blocks = []
    for jj in range(nb):
        j = jj - (nb - 1)
        lag = b - a - 128 * j
        m = (lag >= 0) & (lag < L)
        blk = np.where(m, np.take(np.pad(h[:L], (0, 1)), np.clip(lag, 0, L)), 0.0)
        blocks.append(blk)
    return np.concatenate(blocks, axis=1)


def host_prepare(W1, b1, W2, b2, W3, b3, ar_coef, S_IN, S_OUT, H):
    """Small device tensors in exactly the SBUF layout used."""
    n_s = S_IN // 128
    # W1l[p, k*H + h] = W1[h, k*128 + p]  (lhsT k-tiles for layer 1)
    W1l = np.ascontiguousarray(
        W1.reshape(H, n_s, 128).transpose(2, 1, 0)).reshape(128, -1)
    h, nb = impulse_response(ar_coef, S_OUT)
    b3m = b3.reshape(S_OUT // 128, 128).T                    # [128, n_t]
    # merge the small consts into two blobs (one descriptor each): many
    # small DMAs serialize on their completion semaphores and drag the
    # critical load prefix.
    wb = np.concatenate(
        [np.ascontiguousarray(W2.T), band_blocks(h, nb)], axis=1)
    bias = np.concatenate(
        [b1.reshape(H, 1), b2.reshape(H, 1), b3m], axis=1)
    return {
        "W1l": (W1l * W1_SCALE).astype(F8_NP),  # fp8, scaled into e4m3 range
        "W3l": (np.ascontiguousarray(W3.T) * W1_SCALE).astype(F8_NP),
        "WbB": wb.astype(BF16_NP),              # [128, H + nb*128]: W2l | Hb
        "bias": np.ascontiguousarray(bias, np.float32),  # [128, 2+n_t]
    }, nb


def build_kernel(B_shard, S_IN, S_OUT, H, nb):
    P = 128
    CW = B_shard                  # 1024: one chunk, the full shard
    HW = CW // 2                  # 512: matmul free width (one PSUM bank)
    assert H == P and CW == 1024 and nb == 2
    n_s = S_IN // P               # 32 input k-tiles
    n_t = S_OUT // P              # 32 output t-tiles

    nc = bacc.Bacc(trn_type="TRN2", target_bir_lowering=False, debug=False)

    # x/noise/out are HOST-PRE-TILED to [128, tiles*batch] so every DMA
    # descriptor moves 4KB contiguous per partition (DMA throughput is
    # packet-size bound: ~135GB/s at 1KB chunks vs ~400+ at >=2KB).
    xT_d = nc.dram_tensor("xT", [P, n_s, CW], F8, kind="ExternalInput").ap()
    nT_d = nc.dram_tensor("nT", [P, n_t, CW], BF16,
                          kind="ExternalInput").ap()
    W1_d = nc.dram_tensor("W1l", [P, n_s * H], F8, kind="ExternalInput").ap()
    W3_d = nc.dram_tensor("W3l", [H, S_OUT], F8, kind="ExternalInput").ap()
    Wb_d = nc.dram_tensor("WbB", [P, H + nb * P], BF16,
                          kind="ExternalInput").ap()
    bias_d = nc.dram_tensor("bias", [P, 2 + n_t], F32,
                            kind="ExternalInput").ap()
    out_d = nc.dram_tensor("outT", [P, n_t, CW], BF16,
                           kind="ExternalOutput").ap()

    with tile.TileContext(nc) as tc:
        with tc.tile_pool(name="const", bufs=1) as cpool:
            # the tiny bias blob goes FIRST on sync: the first descriptor
            # of a queue pays the DMA descriptor-path warm-up (~1-2us), so
            # spend it on 17KB instead of on W1/x.
            biass = cpool.tile([P, 2 + n_t], F32, tag="bias")
            nc.sync.dma_start(biass[:], bias_d[:])
            b1s = biass[:, 0:1]
            b2s = biass[:, 1:2]
            b3s = biass[:, 2:]
            W1s = cpool.tile([P, n_s * H], F8, tag="w1")
            nc.sync.dma_start(W1s[:], W1_d[:])

            def W1dr(kk):     # k-pair kk -> [128, 2, H] fp8 lhsT
                return W1s[:, 2 * kk * H:(2 * kk + 2) * H].rearrange(
                    "p (a b) -> p a b", a=2)

            Wbs = cpool.tile([P, H + nb * P], BF16, tag="wb")
            nc.gpsimd.dma_start(Wbs[:], Wb_d[:])
            W2s = Wbs[:, :H]
            Hbs = Wbs[:, H:]

            with (
                tc.tile_pool(name="warm", bufs=1) as wpool,
                tc.tile_pool(name="xT", bufs=8) as xTp,
                tc.tile_pool(name="nT", bufs=4) as nTp,
                tc.tile_pool(name="act", bufs=2) as actp,
                tc.tile_pool(name="th", bufs=3) as thp,
                tc.tile_pool(name="outT", bufs=3) as outp,
                tc.tile_pool(name="psA", bufs=2, space="PSUM") as psA,
                tc.tile_pool(name="psB", bufs=2, space="PSUM") as psB,
            ):
                # ---- x: 4 descriptors of 1MB, ALL on sync right behind
                # W1.  Sync wins wire arbitration, so any x placed on
                # another queue gets starved once sync moves on to noise;
                # the whole ordered critical stream must ride one queue.
                xts = []
                for g in range(n_s // 4):
                    t = xTp.tile([P, 4, CW], F8, tag="xt", name=f"xt{g}")
                    nc.sync.dma_start(t[:],
                                      xT_d[:, g * 4:(g + 1) * 4, :])
                    xts.append(t)
                W3s = cpool.tile([H, S_OUT], F8, tag="w3")
                nc.gpsimd.dma_start(W3s[:], W3_d[:])

                def xdr(kk, h):   # k-pair kk, batch-half h -> [128,2,512] f8
                    g, j = kk // 2, 2 * (kk % 2)
                    return xts[g][:, j:j + 2, h * HW:(h + 1) * HW]

                # ---- noise: 16 descriptors of 512KB (2 t-tiles) on sync
                # strictly behind x; pool backpressure (bufs=3 = 1.5MB
                # eager window) bounds how much noise can interleave into
                # the x phase while the in-loop JIT issue keeps the reload
                # pipeline going without steady-state stalls.
                ntm = []

                def load_n2(g):
                    t = nTp.tile([P, 2, CW], BF16, tag="nt", name=f"nt{g}")
                    nc.sync.dma_start(t[:], nT_d[:, g * 2:(g + 1) * 2, :])
                    ntm.append(t)

                for g in range(4):
                    load_n2(g)

                def nt(m, h):
                    return ntm[m // 2][:, m % 2, h * HW:(h + 1) * HW]

                # ---- PE warm-up: the HAM clock gate defaults the PE array
                # to 1.2 GHz and releases 2.4 GHz only after ~3.4us of
                # sustained matmul activity; warm on a zeroed scratch tile
                # while the first x groups stream in.
                wsrc = wpool.tile([P, 4 * P], BF16, tag="wsrc")
                nc.vector.memset(wsrc[:], 0.0)
                wsnk = wpool.tile([P, 4], F32, tag="wsnk")
                psw = psB.tile([P, CW], F32, tag="ps", name="psw")
                for i in range(10):
                    nc.tensor.matmul(psw[:, :HW], wsrc[:, :P], wsrc[:])
                nc.vector.tensor_copy(wsnk[:], psw[:, :4])

                # ---- L1: 16 DoubleRow fp8 matmuls per batch half.
                psh1 = psA.tile([H, CW], F32, tag="psA", name="psh1")
                for kk in range(n_s // 2):
                    for h in range(2):
                        nc.tensor.matmul(
                            psh1[:, h * HW:(h + 1) * HW], W1dr(kk), xdr(kk, h),
                            start=(kk == 0), stop=(kk == n_s // 2 - 1),
                            perf_mode=DR,
                        )
                h1T = actp.tile([H, CW], BF16, tag="act", name="h1T")
                # scale undoes the x64 put on W1 to lift fp8 denormals
                nc.scalar.activation(
                    h1T[:], psh1[:], mybir.ActivationFunctionType.Relu,
                    bias=b1s, scale=1.0 / W1_SCALE,
                )
                psh2 = psA.tile([H, CW], F32, tag="psA", name="psh2")
                for h in range(2):
                    nc.tensor.matmul(psh2[:, h * HW:(h + 1) * HW], W2s,
                                     h1T[:, h * HW:(h + 1) * HW])
                h2T = actp.tile([H, CW], BF16, tag="act", name="h2T")
                nc.scalar.activation(
                    h2T[:], psh2[:], mybir.ActivationFunctionType.Relu,
                    bias=b2s,
                )

                # ---- t-loop: conv + W3 matmuls (512-wide halves, grouped
                # by stationary), 1024-wide tanh ACT and DVE add, 2-tile
                # merged stores on gpsimd.
                for m in range(n_t):
                    if m % 2 == 0 and m // 2 + 4 < n_t // 2:
                        load_n2(m // 2 + 4)
                    jlist = [j for j in range(-(nb - 1), 1) if m + j >= 0]
                    psc = psB.tile([P, CW], F32, tag="ps", name=f"psc{m}")
                    for i, j in enumerate(jlist):
                        jj = j + nb - 1
                        for h in range(2):
                            nc.tensor.matmul(
                                psc[:, h * HW:(h + 1) * HW],
                                Hbs[:, jj * P:(jj + 1) * P], nt(m + j, h),
                                start=(i == 0), stop=(i == len(jlist) - 1),
                            )
                    psm = psA.tile([P, CW], F32, tag="psA", name=f"psm{m}")
                    for h in range(2):
                        nc.tensor.matmul(
                            psm[:, h * HW:(h + 1) * HW],
                            W3s[:, m * P:(m + 1) * P],
                            h2T[:, h * HW:(h + 1) * HW],
                        )
                    if m % 2 == 0:
                        ot = outp.tile([P, 2, CW], BF16, tag="ot",
                                       name=f"ot{m // 2}")
                    if m == n_t - 1:
                        # finer-grained tail: per-half ACT/add, then store
                        # the two final tiles separately so the drain after
                        # the last matmul is shorter.
                        th = thp.tile([P, CW], BF16, tag="th")
                        for h in range(2):
                            hs = slice(h * HW, (h + 1) * HW)
                            nc.scalar.activation(
                                th[:, hs], psm[:, hs],
                                mybir.ActivationFunctionType.Tanh,
                                bias=b3s[:, m:m + 1], scale=1.0 / W1_SCALE,
                            )
                            nc.vector.tensor_add(ot[:, 1, hs], th[:, hs],
                                                 psc[:, hs])
                            if h == 0:
                                nc.gpsimd.dma_start(
                                    out_d[:, m - 1:m, :], ot[:, 0:1, :])
                        nc.gpsimd.dma_start(out_d[:, m:m + 1, :],
                                            ot[:, 1:2, :])
                        continue
                    th = thp.tile([P, CW], BF16, tag="th")
                    nc.scalar.activation(
                        th[:], psm[:], mybir.ActivationFunctionType.Tanh,
                        bias=b3s[:, m:m + 1], scale=1.0 / W1_SCALE,
                    )
                    nc.vector.tensor_add(ot[:, m % 2, :], th[:], psc[:])
                    if m % 2 == 1:
                        # all stores on gpsimd: on sync they would delay the
                        # in-loop noise reloads in queue-FIFO order.
                        nc.gpsimd.dma_start(out_d[:, m - 1:m + 1, :], ot[:])

    nc.compile()
    return nc


# ---------------------------------------------------------------------------
# Self-contained kernel() entry point (the graded contract).
# ---------------------------------------------------------------------------

N_CORES = 8
_B, _S_IN, _S_OUT, _H, _AR = 8192, 4096, 4096, 128, 7

_CACHE = {}


def _prep_and_build(inputs):
    dev, nb = host_prepare(
        np.asarray(inputs["W1"], np.float32), np.asarray(inputs["b1"], np.float32),
        np.asarray(inputs["W2"], np.float32), np.asarray(inputs["b2"], np.float32),
        np.asarray(inputs["W3"], np.float32), np.asarray(inputs["b3"], np.float32),
        np.asarray(inputs["ar_coef"], np.float32),
        _S_IN, _S_OUT, _H,
    )
    B_total = inputs["x"].shape[0]
    B_shard = B_total // N_CORES
    key = (B_shard, nb)
    if key not in _CACHE:
        _CACHE[key] = build_kernel(B_shard, _S_IN, _S_OUT, _H, nb)
    return _CACHE[key], dev, B_shard


def _tiled_T(a, dt):
    """[B_shard, S] -> [128, (S//128)*B_shard]: host pre-tiling so device
    DMA descriptors move long contiguous per-partition chunks."""
    s = a.shape[1]
    aT = a.astype(dt).T                           # [S, B_shard]
    return np.ascontiguousarray(
        aT.reshape(s // 128, 128, a.shape[0]).transpose(1, 0, 2))


def _in_maps(inputs, dev, B_shard):
    x = np.asarray(inputs["x"], np.float32)
    noise_m = np.asarray(inputs["noise"], np.float32).copy()
    noise_m[:, :_AR] = 0.0
    maps = []
    for c in range(N_CORES):
        sl = slice(c * B_shard, (c + 1) * B_shard)
        m = {"xT": _tiled_T(x[sl], F8_NP),
             "nT": _tiled_T(noise_m[sl], BF16_NP)}
        m.update(dev)
        maps.append(m)
    return maps


def kernel(**inputs):
    nc, dev, B_shard = _prep_and_build(inputs)
    maps = _in_maps(inputs, dev, B_shard)
    import concourse.bass_utils as bass_utils

    res = bass_utils.run_bass_kernel_spmd(
        nc, maps, core_ids=list(range(N_CORES)), trace=False
    )
    shards = []
    for c in range(N_CORES):
        o = np.asarray(res.results[c]["outT"])    # [128, n_t, B] bf16
        s_out = o.shape[1] * 128
        shards.append(o.transpose(1, 0, 2).reshape(s_out, -1).T)
    return np.concatenate(shards, axis=0).astype(np.float32)


def run_traced(inputs):
    """Profiled run (NTFF -> neuron-profile) for the local test harness."""
    import contextlib
    import ctypes
    import sys as _sys
    import types as _types

    so = "/opt/axon/libaxon_pjrt.so"
    if "antenv.axon_hooks" not in _sys.modules:
        try:
            lib2 = ctypes.CDLL(so)
            lib2.axon_start_nrt_profile.argtypes = [
                ctypes.POINTER(ctypes.c_int64), ctypes.c_size_t]
            lib2.axon_start_nrt_profile.restype = ctypes.c_int64
            lib2.axon_stop_nrt_profile.argtypes = [ctypes.c_char_p]
            lib2.axon_stop_nrt_profile.restype = ctypes.c_int64

            @contextlib.contextmanager
            def _hook(output_dir, device_ids):
                import jax
                jax.devices()
                if device_ids:
                    ids_arr = (ctypes.c_int64 * len(device_ids))(*device_ids)
                    rc = lib2.axon_start_nrt_profile(ids_arr, len(device_ids))
                else:
                    rc = lib2.axon_start_nrt_profile(None, 0)
                if rc != 0:
                    raise RuntimeError(f"axon_start_nrt_profile rc={rc}")
                try:
                    yield
                finally:
                    lib2.axon_stop_nrt_profile(str(output_dir).encode())

            mod = _types.ModuleType("antenv.axon_hooks")
            mod.get_axon_ntff_profile_hook = lambda: _hook
            mod.set_axon_ntff_profile_hook = lambda h: None
            _sys.modules["antenv.axon_hooks"] = mod
        except OSError:
            pass
    import concourse.bass_utils as bass_utils
    bass_utils.upload_artifacts = lambda tmpdir: tmpdir

    nc, dev, B_shard = _prep_and_build(inputs)
    maps = _in_maps(inputs, dev, B_shard)
    return bass_utils.run_bass_kernel_spmd(
        nc, maps, core_ids=list(range(N_CORES)), trace=True, trace_cores=[0]
    )
